# revision 4
# baseline (speedup 1.0000x reference)
"""DopDense forward: relu(x @ (w * mult) + b) on 8 trn2 NeuronCores.

Key algebra: w_new = w * mult (per-column scaling) commutes with the matmul,
so out = relu((x @ w) * mult[None, :] + b).  We compute y^T tiles (units on
partitions, batch on free axis) so the per-column mult/bias become
per-partition scale/bias of a fused Relu eviction (scalar-engine activation
or a 2-op vector tensor_scalar).

Sharding: data-parallel over the batch axis (8192 rows/core); w, dop state
replicated.  mult is computed on-device from w, dop_weights_old, indicator,
batch_ctr; the big matmul runs in bf16, everything else fp32.

DMA: the kernel is memory-bound (~26 MB/core), so traffic is spread across
the sync HWDGE queue (x input), the scalar HWDGE queue (most output), and
the gpsimd SWDGE queue (aux inputs + tail output).
"""

import numpy as np
import ml_dtypes

import concourse.bass as bass
import concourse.mybir as mybir
import concourse.tile as tile
from concourse import bacc
from concourse.bass_utils import run_bass_kernel_spmd

F32 = mybir.dt.float32
BF16 = mybir.dt.bfloat16
AF = mybir.ActivationFunctionType
ALU = mybir.AluOpType
BF16_NP = np.dtype(ml_dtypes.bfloat16)

N_CORES = 8
B = 65536
NIN = 512
UNITS = 512
N_DOP = 128
SHARD = B // N_CORES          # 8192 batch rows per core
W = 1024                      # batch window per psum tile (2 PSUM banks)
NWP = SHARD // W              # 8 windows per core
KC = NIN // 128               # 4 contraction chunks
CC = UNITS // 128             # 4 unit chunks
THRESHOLD = 0.0
REF_PERIOD = 2.0

# Static dopaminergic-column index math (mirrors reference.py exactly)
DOP_IDX = np.linspace(1, UNITS - 1, N_DOP, dtype=np.int32)
LEFT_OK = ~np.isin(DOP_IDX - 1, DOP_IDX)
RIGHT_OK = ~np.isin(DOP_IDX + 1, DOP_IDX)
LCOL = (DOP_IDX - 1) % UNITS
RCOL = (DOP_IDX + 1) % UNITS


def _static_masks():
    lmat = np.zeros((CC, 128, 128), np.float32)   # lmat[cc,j,m]: LCOL[j]==cc*128+m
    rmat = np.zeros((CC, 128, 128), np.float32)
    for j in range(N_DOP):
        lmat[LCOL[j] // 128, j, LCOL[j] % 128] = 1.0
        rmat[RCOL[j] // 128, j, RCOL[j] % 128] = 1.0
    return lmat, rmat


LMAT, RMAT = _static_masks()
LOK10 = (LEFT_OK.astype(np.float32) * np.float32(10.0 / NIN)).reshape(128, 1)
ROK10 = (RIGHT_OK.astype(np.float32) * np.float32(10.0 / NIN)).reshape(128, 1)

_CACHED_NC = None


def build_nc():
    global _CACHED_NC
    if _CACHED_NC is not None:
        return _CACHED_NC
    nc = bacc.Bacc("TRN2", target_bir_lowering=False, debug=False,
                   num_swdge_queues=2)

    xt = nc.dram_tensor("xt", [NWP, 128, KC * W], BF16, kind="ExternalInput")
    wkb = nc.dram_tensor("wkb", [KC, CC, 128, 128], BF16, kind="ExternalInput")
    wd = nc.dram_tensor("wd", [128, NIN], F32, kind="ExternalInput")
    od = nc.dram_tensor("od", [128, NIN], F32, kind="ExternalInput")
    lmat = nc.dram_tensor("lmat", [CC, 128, 128], F32, kind="ExternalInput")
    rmat = nc.dram_tensor("rmat", [CC, 128, 128], F32, kind="ExternalInput")
    lok10 = nc.dram_tensor("lok10", [128, 1], F32, kind="ExternalInput")
    rok10 = nc.dram_tensor("rok10", [128, 1], F32, kind="ExternalInput")
    indf = nc.dram_tensor("indf", [128, 1], F32, kind="ExternalInput")
    bctr = nc.dram_tensor("bctr", [128, 1], F32, kind="ExternalInput")
    bias = nc.dram_tensor("bias", [CC, 128, 1], F32, kind="ExternalInput")
    yt = nc.dram_tensor("yt", [NWP, 128, CC * W], F32, kind="ExternalOutput")

    with tile.TileContext(nc) as tc:
        with (
            tc.tile_pool(name="const", bufs=1) as const,
            tc.tile_pool(name="aux", bufs=1) as aux,
            tc.tile_pool(name="xa", bufs=3) as xpool,
            tc.tile_pool(name="ob", bufs=2) as opool,
            tc.tile_pool(name="tmp", bufs=2) as tpool,
            tc.tile_pool(name="ps", bufs=3, space="PSUM") as pspool,
            tc.tile_pool(name="psx", bufs=1, space="PSUM") as psaux,
        ):
            # ---------- aux inputs via gpsimd SWDGE (parallel queue) ----------
            wd_sb = aux.tile([128, NIN], F32, tag="wdld")
            nc.gpsimd.dma_start(wd_sb[:], wd[:])
            od_sb = aux.tile([128, NIN], F32, tag="odld")
            nc.gpsimd.dma_start(od_sb[:], od[:])
            lm, rm = [], []
            for cc in range(CC):
                l = const.tile([128, 128], F32, tag=f"lm{cc}")
                nc.gpsimd.dma_start(l[:], lmat[cc])
                lm.append(l)
                r = const.tile([128, 128], F32, tag=f"rm{cc}")
                nc.gpsimd.dma_start(r[:], rmat[cc])
                rm.append(r)
            lok_sb = const.tile([128, 1], F32, tag="lok")
            nc.gpsimd.dma_start(lok_sb[:], lok10[:])
            rok_sb = const.tile([128, 1], F32, tag="rok")
            nc.gpsimd.dma_start(rok_sb[:], rok10[:])
            ind_sb = const.tile([128, 1], F32, tag="ind")
            nc.gpsimd.dma_start(ind_sb[:], indf[:])
            bc_sb = const.tile([128, 1], F32, tag="bc")
            nc.gpsimd.dma_start(bc_sb[:], bctr[:])
            b_sb = []
            for cc in range(CC):
                bt = const.tile([128, 1], F32, tag=f"b{cc}")
                nc.gpsimd.dma_start(bt[:], bias[cc])
                b_sb.append(bt)

            # weights (small) on the scalar HWDGE queue, ahead of outputs
            wk_sb = []
            for k in range(KC):
                row = []
                for c in range(CC):
                    wkt = const.tile([128, 128], BF16, tag=f"wk{k}_{c}")
                    nc.scalar.dma_start(wkt[:], wkb[k, c])
                    row.append(wkt)
                wk_sb.append(row)

            # ---------- aux compute: dd[j] = sum_i |w[i,d_j] - old[i,d_j]| ----
            dch = aux.tile([128, NIN], F32, tag="dch")
            nc.vector.tensor_tensor(dch[:], wd_sb[:], od_sb[:], op=ALU.subtract)
            dd = const.tile([128, 1], F32, tag="dd")
            nc.vector.tensor_reduce(
                dd[:], dch[:], axis=mybir.AxisListType.X, op=ALU.add,
                apply_absolute_value=True,
            )
            # active = (dd > THRESHOLD) & ((batch_ctr - indicator) > REF_PERIOD)
            t1 = const.tile([128, 1], F32, tag="t1")
            nc.vector.tensor_tensor(t1[:], bc_sb[:], ind_sb[:], op=ALU.subtract)
            c2 = const.tile([128, 1], F32, tag="c2")
            nc.vector.tensor_scalar(c2[:], t1[:], REF_PERIOD, None, op0=ALU.is_gt)
            c1 = const.tile([128, 1], F32, tag="c1")
            nc.vector.tensor_scalar(c1[:], dd[:], THRESHOLD, None, op0=ALU.is_gt)
            av = const.tile([128, 1], F32, tag="av")
            nc.vector.tensor_tensor(av[:], c1[:], c2[:], op=ALU.mult)
            da = const.tile([128, 1], F32, tag="da")
            nc.vector.tensor_tensor(da[:], dd[:], av[:], op=ALU.mult)
            # lfm1 = (fac-1) gated by active & neighbor-ok = 10/512*colsum_d*gate
            lf1 = const.tile([128, 1], F32, tag="lf1")
            nc.vector.tensor_tensor(lf1[:], da[:], lok_sb[:], op=ALU.mult)
            rf1 = const.tile([128, 1], F32, tag="rf1")
            nc.vector.tensor_tensor(rf1[:], da[:], rok_sb[:], op=ALU.mult)
            llf = const.tile([128, 1], F32, tag="llf")
            nc.scalar.activation(llf[:], lf1[:], AF.Ln, bias=1.0)
            lrf = const.tile([128, 1], F32, tag="lrf")
            nc.scalar.activation(lrf[:], rf1[:], AF.Ln, bias=1.0)

            # multiplicative scatter to columns via log-space accumulate
            mult_sb = []
            for cc in range(CC):
                ml_ps = psaux.tile([128, 1], F32, tag="auxps")
                nc.tensor.matmul(ml_ps[:], lm[cc][:], llf[:], start=True, stop=False)
                nc.tensor.matmul(ml_ps[:], rm[cc][:], lrf[:], start=False, stop=True)
                m = const.tile([128, 1], F32, tag=f"mult{cc}")
                nc.scalar.activation(m[:], ml_ps[:], AF.Exp)
                mult_sb.append(m)

            # ---------- main: y^T = (w^T x^T) scaled+biased+relu ----------
            for wp in range(NWP):
                xa = xpool.tile([128, KC * W], BF16, tag="xa")
                nc.sync.dma_start(xa[:], xt[wp])
                ob = opool.tile([128, CC * W], F32, tag="ob")
                for c in range(CC):
                    ps = pspool.tile([128, W], F32, tag="mps")
                    for k in range(KC):
                        for s in range(W // 512):
                            nc.tensor.matmul(
                                ps[:, s * 512:(s + 1) * 512],
                                wk_sb[k][c][:],
                                xa[:, k * W + s * 512: k * W + (s + 1) * 512],
                                start=(k == 0), stop=(k == KC - 1),
                            )
                    obs = ob[:, c * W:(c + 1) * W]
                    if c < 3:
                        nc.scalar.activation(obs, ps[:], AF.Relu,
                                             bias=b_sb[c][:], scale=mult_sb[c][:])
                    else:
                        tmp = tpool.tile([128, W], F32, tag="evt")
                        nc.vector.tensor_scalar(
                            tmp[:], ps[:], mult_sb[c][:], b_sb[c][:],
                            op0=ALU.mult, op1=ALU.add)
                        nc.vector.tensor_scalar(
                            obs, tmp[:], 0.0, None, op0=ALU.max)
                # output: most on the scalar HWDGE queue, tail on gpsimd SWDGE
                if wp < 6:
                    nc.scalar.dma_start(yt[wp], ob[:])
                else:
                    nc.gpsimd.dma_start(yt[wp], ob[:])

    nc.compile()
    _CACHED_NC = nc
    return nc


LAST_RESULTS = None


def kernel(x, w, b, dop_weights_old, indicator, batch_ctr):
    global LAST_RESULTS
    x = np.asarray(x, dtype=np.float32)
    w = np.ascontiguousarray(np.asarray(w, dtype=np.float32))
    b_arr = np.asarray(b, dtype=np.float32)
    old = np.asarray(dop_weights_old, dtype=np.float32)
    ind = np.asarray(indicator, dtype=np.float32)
    bc_val = float(np.asarray(batch_ctr).item())

    nc = build_nc()

    # replicated (per-core identical) inputs; all reshapes/gathers are pure
    # data marshaling -- every arithmetic op happens on device
    wkb = np.ascontiguousarray(
        w.reshape(KC, 128, CC, 128).transpose(0, 2, 1, 3)).astype(BF16_NP)
    wT = w.T
    oT = old.T
    wd = np.ascontiguousarray(wT[DOP_IDX])          # [128, 512] dop columns
    od = np.ascontiguousarray(oT[DOP_IDX])
    bias = np.ascontiguousarray(b_arr.reshape(CC, 128, 1))
    indf = np.ascontiguousarray(ind.reshape(128, 1))
    bctr = np.full((128, 1), bc_val, np.float32)

    common = dict(
        wkb=wkb, wd=wd, od=od, lmat=LMAT, rmat=RMAT,
        lok10=LOK10, rok10=ROK10, indf=indf, bctr=bctr, bias=bias,
    )

    xbf = x.astype(BF16_NP)
    in_maps = []
    for i in range(N_CORES):
        xs = xbf[i * SHARD:(i + 1) * SHARD]          # [8192, 512]
        xtc = np.ascontiguousarray(
            xs.reshape(NWP, W, KC, 128).transpose(0, 3, 2, 1)
        ).reshape(NWP, 128, KC * W)
        in_maps.append(dict(common, xt=xtc))

    res = run_bass_kernel_spmd(nc, in_maps, core_ids=list(range(N_CORES)))
    LAST_RESULTS = res

    out = np.empty((B, UNITS), np.float32)
    for i in range(N_CORES):
        ytc = res.results[i]["yt"].reshape(NWP, 128, CC, W)
        out[i * SHARD:(i + 1) * SHARD] = (
            ytc.transpose(0, 3, 2, 1).reshape(SHARD, UNITS))
    return out


# revision 8
# speedup vs baseline: 1.1300x; 1.1300x over previous
"""DopDense forward: relu(x @ (w * mult) + b) on 8 trn2 NeuronCores.

Key algebra: w_new = w * mult (per-column scaling) commutes with the matmul,
so out = relu((x @ w) * mult[None, :] + b).  We compute y^T tiles (units on
partitions, batch on free axis) so the per-column mult/bias become
per-partition scale/bias of a fused Relu eviction (scalar-engine activation
or a 2-op vector tensor_scalar).

Sharding: data-parallel over the batch axis (8192 rows/core); w, dop state
replicated.  mult is computed on-device from w, dop_weights_old, indicator,
batch_ctr; the big matmul runs in bf16, everything else fp32.

The kernel is memory-bound (~26 MB/core), so DMA traffic is spread across
the sync HWDGE, scalar HWDGE and gpsimd SWDGE queues, with few large DMAs
(each DMA issue costs ~650ns of engine time).
"""

import numpy as np
import ml_dtypes

import concourse.bass as bass
import concourse.mybir as mybir
import concourse.tile as tile
from concourse import bacc
from concourse.bass_utils import run_bass_kernel_spmd

F32 = mybir.dt.float32
BF16 = mybir.dt.bfloat16
AF = mybir.ActivationFunctionType
ALU = mybir.AluOpType
BF16_NP = np.dtype(ml_dtypes.bfloat16)

N_CORES = 8
B = 65536
NIN = 512
UNITS = 512
N_DOP = 128
SHARD = B // N_CORES          # 8192 batch rows per core
W = 1024                      # batch window per psum tile (2 PSUM banks)
NWP = SHARD // W              # 8 windows per core
KC = NIN // 128               # 4 contraction chunks
CC = UNITS // 128             # 4 unit chunks
THRESHOLD = 0.0
REF_PERIOD = 2.0

# Static dopaminergic-column index math (mirrors reference.py exactly)
DOP_IDX = np.linspace(1, UNITS - 1, N_DOP, dtype=np.int32)
LEFT_OK = ~np.isin(DOP_IDX - 1, DOP_IDX)
RIGHT_OK = ~np.isin(DOP_IDX + 1, DOP_IDX)
LCOL = (DOP_IDX - 1) % UNITS
RCOL = (DOP_IDX + 1) % UNITS


def _static_masks():
    # scatter matrices, packed side by side: [j, cc*128+m] for L, then R
    lr = np.zeros((128, 2 * UNITS), np.float32)
    for j in range(N_DOP):
        lr[j, LCOL[j]] = 1.0
        lr[j, UNITS + RCOL[j]] = 1.0
    return lr


LRMAT = _static_masks()
LOK10 = LEFT_OK.astype(np.float32) * np.float32(10.0 / NIN)
ROK10 = RIGHT_OK.astype(np.float32) * np.float32(10.0 / NIN)

_CACHED_NC = None


def build_nc():
    global _CACHED_NC
    if _CACHED_NC is not None:
        return _CACHED_NC
    nc = bacc.Bacc("TRN2", target_bir_lowering=False, debug=False,
                   num_swdge_queues=2)

    xt = nc.dram_tensor("xt", [NWP, 128, KC * W], BF16, kind="ExternalInput")
    # w chunks packed as [128, (k*CC+c)*128 + m] (bf16, matmul stationary)
    wkb = nc.dram_tensor("wkb", [128, KC * CC * 128], BF16, kind="ExternalInput")
    # dop columns of w^T / old^T side by side: [128, 0:512]=w, [128, 512:1024]=old
    wod = nc.dram_tensor("wod", [128, 2 * NIN], F32, kind="ExternalInput")
    lrmat = nc.dram_tensor("lrmat", [128, 2 * UNITS], F32, kind="ExternalInput")
    # 8 per-partition vectors: lok10, rok10, indicator, batch_ctr, b0..b3
    vecs = nc.dram_tensor("vecs", [128, 8], F32, kind="ExternalInput")
    yt = nc.dram_tensor("yt", [NWP, 128, CC * W], F32, kind="ExternalOutput")

    with tile.TileContext(nc) as tc:
        with (
            tc.tile_pool(name="const", bufs=1) as const,
            tc.tile_pool(name="aux", bufs=1) as aux,
            tc.tile_pool(name="xa", bufs=4) as xpool,
            tc.tile_pool(name="ob", bufs=3) as opool,
            tc.tile_pool(name="tmp", bufs=2) as tpool,
        ):
            # ---------- input DMAs: few, large, spread over 3 queues ----------
            wod_sb = aux.tile([128, 2 * NIN], F32, tag="wod")
            nc.gpsimd.dma_start(wod_sb[:], wod[:])
            lr_sb = const.tile([128, 2 * UNITS], F32, tag="lr")
            nc.gpsimd.dma_start(lr_sb[:], lrmat[:])
            v_sb = const.tile([128, 8], F32, tag="v")
            nc.gpsimd.dma_start(v_sb[:], vecs[:])
            wk_sb = const.tile([128, KC * CC * 128], BF16, tag="wk")
            nc.scalar.dma_start(wk_sb[:], wkb[:])

            def wk_tile(k, c):
                i = k * CC + c
                return wk_sb[:, i * 128:(i + 1) * 128]

            # x windows: wp0 split for fast start; then alternate sync/gpsimd.
            # Rolling prefetch (depth 3) so queue order matches readiness.
            xa_tiles = {}

            def load_xa(wp):
                xa = xpool.tile([128, KC * W], BF16, tag="xa")
                if wp == 0:
                    nc.sync.dma_start(xa[:, :2 * W], xt[0][:, :2 * W])
                    nc.scalar.dma_start(xa[:, 2 * W:], xt[0][:, 2 * W:])
                elif wp % 2 == 1:
                    nc.gpsimd.dma_start(xa[:], xt[wp])
                else:
                    nc.sync.dma_start(xa[:], xt[wp])
                xa_tiles[wp] = xa

            for wp in range(3):
                load_xa(wp)

            # ---------- aux compute: dd[j] = sum_i |w[i,d_j] - old[i,d_j]| ----
            dch = aux.tile([128, NIN], F32, tag="dch")
            nc.vector.tensor_tensor(dch[:], wod_sb[:, :NIN], wod_sb[:, NIN:],
                                    op=ALU.subtract)
            dd = const.tile([128, 1], F32, tag="dd")
            nc.vector.tensor_reduce(
                dd[:], dch[:], axis=mybir.AxisListType.X, op=ALU.add,
                apply_absolute_value=True,
            )
            # active = (dd > THRESHOLD) & ((batch_ctr - indicator) > REF_PERIOD)
            t1 = const.tile([128, 1], F32, tag="t1")
            nc.vector.tensor_tensor(t1[:], v_sb[:, 3:4], v_sb[:, 2:3],
                                    op=ALU.subtract)
            c2 = const.tile([128, 1], F32, tag="c2")
            nc.vector.tensor_scalar(c2[:], t1[:], REF_PERIOD, None, op0=ALU.is_gt)
            c1 = const.tile([128, 1], F32, tag="c1")
            nc.vector.tensor_scalar(c1[:], dd[:], THRESHOLD, None, op0=ALU.is_gt)
            av = const.tile([128, 1], F32, tag="av")
            nc.vector.tensor_tensor(av[:], c1[:], c2[:], op=ALU.mult)
            da = const.tile([128, 1], F32, tag="da")
            nc.vector.tensor_tensor(da[:], dd[:], av[:], op=ALU.mult)
            lf1 = const.tile([128, 1], F32, tag="lf1")
            nc.vector.tensor_tensor(lf1[:], da[:], v_sb[:, 0:1], op=ALU.mult)
            rf1 = const.tile([128, 1], F32, tag="rf1")
            nc.vector.tensor_tensor(rf1[:], da[:], v_sb[:, 1:2], op=ALU.mult)
            llf = const.tile([128, 1], F32, tag="llf")
            nc.scalar.activation(llf[:], lf1[:], AF.Ln, bias=1.0)
            lrf = const.tile([128, 1], F32, tag="lrf")
            nc.scalar.activation(lrf[:], rf1[:], AF.Ln, bias=1.0)

            # multiplicative scatter to columns via log-space accumulate
            mult_sb = []
            with tc.tile_pool(name="psx", bufs=1, space="PSUM") as psaux:
                for cc in range(CC):
                    ml_ps = psaux.tile([128, 1], F32, tag="auxps")
                    nc.tensor.matmul(ml_ps[:], lr_sb[:, cc * 128:(cc + 1) * 128],
                                     llf[:], start=True, stop=False)
                    nc.tensor.matmul(ml_ps[:],
                                     lr_sb[:, UNITS + cc * 128:UNITS + (cc + 1) * 128],
                                     lrf[:], start=False, stop=True)
                    m = const.tile([128, 1], F32, tag=f"mult{cc}")
                    nc.scalar.activation(m[:], ml_ps[:], AF.Exp)
                    mult_sb.append(m)

            # ---------- main: y^T = (w^T x^T) scaled+biased+relu ----------
            with tc.tile_pool(name="ps", bufs=4, space="PSUM") as pspool:
                for wp in range(NWP):
                    if wp + 3 < NWP:
                        load_xa(wp + 3)
                    xa = xa_tiles[wp]
                    ob = opool.tile([128, CC * W], F32, tag="ob")
                    for c in range(CC):
                        ps = pspool.tile([128, W], F32, tag="mps")
                        for k in range(KC):
                            for s in range(W // 512):
                                nc.tensor.matmul(
                                    ps[:, s * 512:(s + 1) * 512],
                                    wk_tile(k, c),
                                    xa[:, k * W + s * 512: k * W + (s + 1) * 512],
                                    start=(k == 0), stop=(k == KC - 1),
                                )
                        obs = ob[:, c * W:(c + 1) * W]
                        if c < 3:
                            nc.scalar.activation(
                                obs, ps[:], AF.Relu,
                                bias=v_sb[:, 4 + c:5 + c], scale=mult_sb[c][:])
                        else:
                            tmp = tpool.tile([128, W], F32, tag="evt")
                            nc.vector.tensor_scalar(
                                tmp[:], ps[:], mult_sb[c][:], v_sb[:, 4 + c:5 + c],
                                op0=ALU.mult, op1=ALU.add)
                            nc.vector.tensor_scalar(
                                obs, tmp[:], 0.0, None, op0=ALU.max)
                        # drain output halves as soon as each pair is evicted
                        if c == 1:
                            nc.scalar.dma_start(yt[wp][:, :2 * W], ob[:, :2 * W])
                        elif c == 3:
                            nc.sync.dma_start(yt[wp][:, 2 * W:], ob[:, 2 * W:])

    nc.compile()
    _CACHED_NC = nc
    return nc


LAST_RESULTS = None


def kernel(x, w, b, dop_weights_old, indicator, batch_ctr):
    global LAST_RESULTS
    x = np.asarray(x, dtype=np.float32)
    w = np.ascontiguousarray(np.asarray(w, dtype=np.float32))
    b_arr = np.asarray(b, dtype=np.float32)
    old = np.asarray(dop_weights_old, dtype=np.float32)
    ind = np.asarray(indicator, dtype=np.float32)
    bc_val = float(np.asarray(batch_ctr).item())

    nc = build_nc()

    # replicated (per-core identical) inputs; all reshapes/gathers are pure
    # data marshaling -- every arithmetic op happens on device
    wkb = np.ascontiguousarray(
        w.reshape(KC, 128, CC, 128).transpose(1, 0, 2, 3)
    ).reshape(128, KC * CC * 128).astype(BF16_NP)
    wod = np.concatenate([w.T[DOP_IDX], old.T[DOP_IDX]], axis=1)
    wod = np.ascontiguousarray(wod)
    vecs = np.stack(
        [LOK10, ROK10, ind.astype(np.float32),
         np.full(128, bc_val, np.float32)]
        + [b_arr[c * 128:(c + 1) * 128] for c in range(CC)], axis=1)
    vecs = np.ascontiguousarray(vecs.astype(np.float32))

    common = dict(wkb=wkb, wod=wod, lrmat=LRMAT, vecs=vecs)

    xbf = x.astype(BF16_NP)
    in_maps = []
    for i in range(N_CORES):
        xs = xbf[i * SHARD:(i + 1) * SHARD]          # [8192, 512]
        xtc = np.ascontiguousarray(
            xs.reshape(NWP, W, KC, 128).transpose(0, 3, 2, 1)
        ).reshape(NWP, 128, KC * W)
        in_maps.append(dict(common, xt=xtc))

    res = run_bass_kernel_spmd(nc, in_maps, core_ids=list(range(N_CORES)))
    LAST_RESULTS = res

    out = np.empty((B, UNITS), np.float32)
    for i in range(N_CORES):
        ytc = res.results[i]["yt"].reshape(NWP, 128, CC, W)
        out[i * SHARD:(i + 1) * SHARD] = (
            ytc.transpose(0, 3, 2, 1).reshape(SHARD, UNITS))
    return out
